# revision 9
# baseline (speedup 1.0000x reference)
"""BandSplit layer Trainium2 kernel.

Computes, for input [16, 1000, 257]:
  - 28 frequency bands: 8 bands x 4 bins (bins 0..31), 12 x 8 (32..127),
    8 x 16 (128..255)  (bin 256 unused)
  - per-band layernorm over the band's bins (eps=1e-3), with per-band
    gamma/beta, then a per-band dense projection [c] -> [128] plus bias.
  - output [16, 1000, 28, 128]

Strategy: data-parallel over batch across 8 NeuronCores (2 batches =
2000 tokens per core).  gamma is folded into the dense weights and
beta/bias into a single per-output bias on the host, so the device does
plain layernorm + matmul.

DMA choreography (the kernel is HBM-write bound at ~400 GB/s/core):
  - all of x is prefetched with 5 batched DMAs (tile 0 first), weights
    are packed densely per K-group (0.5 MB instead of a 1.8 MB mostly
    zero block-diagonal matrix) and the matmuls contract over partition
    subranges of the transposed activations,
  - the scalar (Activation) HWDGE queue carries all loads, the sync
    queue carries only output stores (2 half-tile stores per 128-token
    tile) so the store stream is never stuck behind load issues.
Per 128-token tile: LN stats via free-dim reduces (DVE) -> rstd via one
Rsqrt -> normalize in place (GpSimd) -> PE transpose to [bins, tok] ->
7 K-sliced fp32r matmuls (N=512) in 2-bank PSUM pairs -> drains spread
over Scalar/Vector/GpSimd -> half-tile DMAs out.
"""

import sys

import numpy as np

for _p in ("/opt/trn_rl_repo", "/root/.axon_site/_ro/trn_rl_repo"):
    if _p not in sys.path:
        sys.path.append(_p)

EPS = 1e-3
D = 128
GROUPS = [(8, 4, 0), (12, 8, 32), (8, 16, 128)]  # (n_bands, bins_per_band, start_bin)
B, T, F = 16, 1000, 257
N_CORES = 8
TOK = B * T // N_CORES  # tokens per core = 2000
NB = sum(n for n, _, _ in GROUPS)  # 28 bands
OUT_COLS = NB * D  # 3584
P = 128
N_CHUNK = 512  # matmul free-dim chunk (one PSUM bank)
N_CHUNKS = OUT_COLS // N_CHUNK  # 7
# Per-band layout: (ktile, krow0, c) per band; ktile 0 = bins 0..127,
# ktile 1 = bins 128..255. Output cols for band i are [i*128, (i+1)*128).
_BANDS = []
for _n, _c, _s in GROUPS:
    for _k in range(_n):
        _bin0 = _s + _k * _c
        _BANDS.append((_bin0 // 128, _bin0 % 128, _c))

# Chunk j covers bands 4j..4j+3 / cols [512j, 512j+512); the matmul
# contracts over a partition subrange of the transposed-activation half
# tile.  HW: operand base partition must be 0/32/64 and lhsT/rhs bases
# must match, so chunk weights sit at the same partition rows of one
# [128, 3584] tile and K is padded to the enclosing 32/64-aligned range
# (padding zeros are packed host-side).
# (xnt half, row0, rows) per chunk; rhs = wfull[row0:row0+rows, 512j:...]
_CHUNKS = [
    (0, 0, 32),
    (0, 0, 32),
    (0, 32, 32),
    (0, 64, 32),
    (0, 64, 64),
    (1, 0, 64),
    (1, 64, 64),
]
# Packed dram tensors and their destinations in the wfull tile:
# wk32 [32, 2048] = chunks 0-3, wk64 [64, 1536] = chunks 4-6.
_WDMA = [  # (wname, src col0, dst row0, dst col0, ncols)
    ("wk32", 0, 0, 0, 1024),
    ("wk32", 1024, 32, 1024, 512),
    ("wk32", 1536, 64, 1536, 512),
    ("wk64", 0, 64, 2048, 512),
    ("wk64", 512, 0, 2560, 512),
    ("wk64", 1024, 64, 3072, 512),
]
_WSHAPES = {"wk32": (32, 2048), "wk64": (64, 1536)}
# x prefetch batches (start tile, ntiles); tile 0 alone so compute can
# start as early as possible. Tile 15 holds only 80 valid tokens.
_XBATCH = [(0, 1), (1, 5), (6, 5), (11, 4), (15, 1)]

_STATE = {}


def _build(has_bias):
    """Trace + compile the Bass kernel (cached per process)."""
    from contextlib import ExitStack

    import concourse.bass as bass
    import concourse.tile as tile
    from concourse import bacc, mybir

    f32 = mybir.dt.float32
    f32r = mybir.dt.float32r
    nc = bacc.Bacc(
        "TRN2", target_bir_lowering=False, debug=False, num_devices=N_CORES
    )
    x_d = nc.dram_tensor("x", [TOK, F], f32, kind="ExternalInput").ap()
    # Declared float32r (same 4-byte layout): DMA straight to the fp32r
    # weight tiles with no on-chip rounding pass.
    w_d = {
        name: nc.dram_tensor(name, list(shape), f32r, kind="ExternalInput").ap()
        for name, shape in _WSHAPES.items()
    }
    id_d = nc.dram_tensor("ident", [P, P], f32, kind="ExternalInput").ap()
    ci_d = nc.dram_tensor("cinv2", [1, 2 * NB], f32, kind="ExternalInput").ap()
    if has_bias:
        b_d = nc.dram_tensor("bias", [1, OUT_COLS], f32, kind="ExternalInput").ap()
    out_d = nc.dram_tensor("out", [TOK, OUT_COLS], f32, kind="ExternalOutput").ap()

    n_tiles = (TOK + P - 1) // P

    with tile.TileContext(nc) as tc, ExitStack() as ctx:
        const = ctx.enter_context(tc.tile_pool(name="const", bufs=1))
        xin = ctx.enter_context(tc.tile_pool(name="xin", bufs=1))
        sqp = ctx.enter_context(tc.tile_pool(name="sqp", bufs=3))
        ln = ctx.enter_context(tc.tile_pool(name="ln", bufs=3))
        xnt = ctx.enter_context(tc.tile_pool(name="xnt", bufs=3))
        outp = ctx.enter_context(tc.tile_pool(name="outp", bufs=5))
        ps_tr = ctx.enter_context(tc.tile_pool(name="ps_tr", bufs=2, space="PSUM"))
        ps_mm = ctx.enter_context(tc.tile_pool(name="ps_mm", bufs=3, space="PSUM"))

        # All of x stays resident (16.4 KB/partition): tile t of 128
        # tokens lives at xall[:, t, :].
        xall = xin.tile([P, n_tiles, F], f32)

        def load_x(t0, nt, engine):
            r0 = t0 * P
            rn = min(nt * P, TOK - r0)
            if rn == nt * P:
                src = x_d[r0 : r0 + rn, :].rearrange("(a p) f -> p a f", p=P)
                engine.dma_start(out=xall[:, t0 : t0 + nt, :], in_=src)
            else:  # trailing partial tile
                engine.dma_start(
                    out=xall[:rn, t0 : t0 + 1, :].rearrange("p a f -> p (a f)"),
                    in_=x_d[r0 : r0 + rn, :],
                )

        # Scalar HWDGE queue: everything tile 0 needs, in need order.
        load_x(0, 1, nc.scalar)
        cinv2 = const.tile([P, 2 * NB], f32)
        nc.scalar.dma_start(
            out=cinv2[:],
            in_=bass.AP(tensor=ci_d.tensor, offset=ci_d.offset, ap=[[0, P], ci_d.ap[1]]),
        )
        ident = const.tile([P, P], f32)
        nc.scalar.dma_start(out=ident[:], in_=id_d)
        wfull = const.tile([P, OUT_COLS], f32r)
        for wname, sc0, dr0, dc0, ncols in _WDMA:
            rows = _WSHAPES[wname][0]
            nc.scalar.dma_start(
                out=wfull[dr0 : dr0 + rows, dc0 : dc0 + ncols],
                in_=w_d[wname][:, sc0 : sc0 + ncols],
            )
        if has_bias:
            bias_sb = const.tile([P, OUT_COLS], f32)
            nc.scalar.dma_start(
                out=bias_sb[:],
                in_=bass.AP(
                    tensor=b_d.tensor, offset=b_d.offset, ap=[[0, P], b_d.ap[1]]
                ),
            )
        # Sync queue: the remaining x batches, then only output stores.
        for t0, nt in _XBATCH[1:]:
            load_x(t0, nt, nc.sync)

        eps_t = const.tile([P, 1], f32)
        nc.vector.memset(eps_t[:], EPS)

        for it in range(n_tiles):
            t0 = it * P
            tn = min(P, TOK - t0)

            xt = xall[:tn, it, :]

            # --- layernorm statistics (per token x band) ---
            sq = sqp.tile([P, 256], f32)
            nc.gpsimd.tensor_mul(sq[:tn, :], xt[:, 0:256], xt[:, 0:256])

            ss = ln.tile([P, 2, NB], f32)
            b0 = 0
            for n, c, s in GROUPS:
                xg = xt[:, s : s + n * c].rearrange("p (g c) -> p g c", g=n)
                sg = sq[:tn, s : s + n * c].rearrange("p (g c) -> p g c", g=n)
                nc.vector.reduce_sum(
                    out=ss[:tn, 0, b0 : b0 + n], in_=xg, axis=mybir.AxisListType.X
                )
                nc.vector.reduce_sum(
                    out=ss[:tn, 1, b0 : b0 + n], in_=sg, axis=mybir.AxisListType.X
                )
                b0 += n

            me = ln.tile([P, 2, NB], f32)  # me[:,0]=mean, me[:,1]=E[x^2]
            nc.vector.tensor_mul(
                me[:tn].rearrange("p a b -> p (a b)"),
                ss[:tn].rearrange("p a b -> p (a b)"),
                cinv2[:tn],
            )
            mean = me[:, 0]
            var = ln.tile([P, NB], f32)
            nc.vector.tensor_mul(var[:tn, :], mean[:tn, :], mean[:tn, :])
            nc.vector.tensor_sub(var[:tn, :], me[:tn, 1, :], var[:tn, :])
            rstd = ln.tile([P, NB], f32)
            nc.scalar.activation(
                out=rstd[:tn, :],
                in_=var[:tn, :],
                func=mybir.ActivationFunctionType.Sqrt,
                bias=eps_t[:tn, :],
                scale=1.0,
            )
            nc.vector.reciprocal(out=rstd[:tn, :], in_=rstd[:tn, :])

            # --- normalize in place: xn = (x - mean) * rstd (GpSimd) ---
            b0 = 0
            for n, c, s in GROUPS:
                xg = xt[:, s : s + n * c].rearrange("p (g c) -> p g c", g=n)
                nc.gpsimd.tensor_sub(
                    xg, xg, mean[:tn, b0 : b0 + n].to_broadcast((tn, n, c))
                )
                nc.gpsimd.tensor_mul(
                    xg, xg, rstd[:tn, b0 : b0 + n].to_broadcast((tn, n, c))
                )
                b0 += n

            # --- transpose to [bins, tok] (two 128-col halves) ---
            xnt_h = []
            for h in range(2):
                pt = ps_tr.tile([P, P], f32, tag="pt")
                nc.tensor.transpose(
                    pt[:, :tn], xt[:, h * P : (h + 1) * P], ident[:tn, :tn]
                )
                st = xnt.tile([P, P], f32r, tag=f"xnt{h}")
                nc.scalar.copy(st[:, :tn], pt[:, :tn])
                xnt_h.append(st)

            # --- 7 K-sliced fp32r matmuls in 2-bank PSUM pairs ---
            # pair drains spread over scalar/vector/gpsimd; one output
            # store per 2 pairs so the sync queue only carries stores.
            ot = outp.tile([P, OUT_COLS], f32)
            drain_eng = [nc.scalar, nc.vector, nc.scalar, nc.vector]
            for pair in range(4):
                js = [j for j in (2 * pair, 2 * pair + 1) if j < N_CHUNKS]
                pm = ps_mm.tile([P, 2 * N_CHUNK], f32, tag="pm")
                for k, j in enumerate(js):
                    h, r0, rows = _CHUNKS[j]
                    nc.tensor.matmul(
                        pm[:tn, k * N_CHUNK : (k + 1) * N_CHUNK],
                        xnt_h[h][r0 : r0 + rows, :tn],
                        wfull[r0 : r0 + rows, j * N_CHUNK : (j + 1) * N_CHUNK],
                        start=True,
                        stop=True,
                    )
                c0 = 2 * pair * N_CHUNK
                c1 = c0 + len(js) * N_CHUNK
                osl = ot[:tn, c0:c1]
                pms = pm[:tn, 0 : (c1 - c0)]
                if has_bias:
                    eng = nc.vector if pair < 2 else nc.gpsimd
                    eng.tensor_add(osl, pms, bias_sb[:tn, c0:c1])
                else:
                    eng = drain_eng[pair]
                    if eng is nc.scalar:
                        eng.copy(osl, pms)
                    else:
                        eng.tensor_copy(osl, pms)
                if pair % 2 == 1:
                    h0 = (pair - 1) * 2 * N_CHUNK
                    nc.sync.dma_start(
                        out=out_d[t0 : t0 + tn, h0:c1], in_=ot[:tn, h0:c1]
                    )

    nc.compile()
    return nc


def _get_nc(has_bias):
    key = ("nc", has_bias)
    if key not in _STATE:
        _STATE[key] = _build(has_bias)
    return _STATE[key]


def _pack_weights(inputs):
    """Fold gamma into W, beta/b into bias; pack 32-aligned K-blocks."""
    packs = {name: np.zeros(shape, dtype=np.float32) for name, shape in _WSHAPES.items()}
    bias = np.zeros((OUT_COLS,), dtype=np.float32)
    bi = 0
    for gi, (n, c, _s) in enumerate(GROUPS, start=1):
        gamma = np.asarray(inputs[f"gamma{gi}"], dtype=np.float32)  # [n, c]
        beta = np.asarray(inputs[f"beta{gi}"], dtype=np.float32)  # [n, c]
        W = np.asarray(inputs[f"W{gi}"], dtype=np.float32)  # [n, c, D]
        b = np.asarray(inputs[f"b{gi}"], dtype=np.float32)  # [n, D]
        for k in range(n):
            chunk, lk = bi // 4, bi % 4  # chunk j holds bands 4j..4j+3
            _h, row0, _rows = _CHUNKS[chunk]
            _kt, krow0, cc = _BANDS[bi]
            assert cc == c
            wname = "wk32" if chunk < 4 else "wk64"
            bcol0 = (chunk if chunk < 4 else chunk - 4) * N_CHUNK
            r0 = krow0 - row0  # band rows inside the chunk's K-block
            packs[wname][
                r0 : r0 + c, bcol0 + lk * D : bcol0 + (lk + 1) * D
            ] = gamma[k][:, None] * W[k]
            bias[bi * D : (bi + 1) * D] = beta[k] @ W[k] + b[k]
            bi += 1
    return packs, bias


def _cinv2():
    ci = np.zeros((1, 2 * NB), dtype=np.float32)
    for half in range(2):
        b0 = 0
        for n, c, _s in GROUPS:
            ci[0, half * NB + b0 : half * NB + b0 + n] = 1.0 / c
            b0 += n
    return ci


def _prepare(inputs):
    """-> (nc, in_maps) for the 8 cores."""
    x = np.asarray(inputs["inputs"], dtype=np.float32)
    assert x.shape == (B, T, F), x.shape
    packs, bias = _pack_weights(inputs)
    has_bias = bool(np.any(bias != 0.0))

    nc = _get_nc(has_bias)

    xflat = np.ascontiguousarray(x.reshape(B * T, F))
    ident = np.eye(P, dtype=np.float32)
    cinv2 = _cinv2()
    in_maps = []
    for c in range(N_CORES):
        m = {
            "x": xflat[c * TOK : (c + 1) * TOK],
            "ident": ident,
            "cinv2": cinv2,
        }
        m.update(packs)
        if has_bias:
            m["bias"] = bias.reshape(1, OUT_COLS)
        in_maps.append(m)
    return nc, in_maps


def kernel(**inputs):
    from concourse.bass_utils import run_bass_kernel_spmd

    nc, in_maps = _prepare(inputs)
    res = run_bass_kernel_spmd(nc, in_maps, list(range(N_CORES))).results
    out = np.concatenate([r["out"] for r in res], axis=0)
    return out.reshape(B, T, NB, D)


# revision 10
# speedup vs baseline: 1.1067x; 1.1067x over previous
"""BandSplit layer Trainium2 kernel.

Computes, for input [16, 1000, 257]:
  - 28 frequency bands: 8 bands x 4 bins (bins 0..31), 12 x 8 (32..127),
    8 x 16 (128..255)  (bin 256 unused)
  - per-band layernorm over the band's bins (eps=1e-3), with per-band
    gamma/beta, then a per-band dense projection [c] -> [128] plus bias.
  - output [16, 1000, 28, 128]

Strategy: data-parallel over batch across 8 NeuronCores (2 batches =
2000 tokens per core).  gamma is folded into the dense weights and
beta/bias into a single per-output bias on the host, so the device does
plain layernorm + matmul.

DMA choreography (the kernel is HBM-write bound at ~400 GB/s/core):
  - x is repacked host-side into the exact SBUF image [128, 16, 257]
    so the prefetch runs in 3 DMAs with multi-KB descriptors (tiles 0-1
    first so compute starts early),
  - the scalar (Activation) HWDGE queue carries all loads, the sync
    queue carries only output stores (2 half-tile stores per 128-token
    tile) so the store stream is never stuck behind load issues,
  - per-band 1/c constants are built with memsets, not DMA.
Per 128-token tile: LN stats via free-dim reduces (DVE) -> normalize in
place (vector for the first tiles to shortcut the startup dependency
chain, GpSimd after) -> PE transpose to [bins, tok] -> 7 fp32r matmuls
(K=128 against the packed block-diagonal weights, N=512) in 2-bank
PSUM pairs -> drains alternating Scalar/Vector -> half-tile DMAs out.
"""

import sys

import numpy as np

for _p in ("/opt/trn_rl_repo", "/root/.axon_site/_ro/trn_rl_repo"):
    if _p not in sys.path:
        sys.path.append(_p)

EPS = 1e-3
D = 128
GROUPS = [(8, 4, 0), (12, 8, 32), (8, 16, 128)]  # (n_bands, bins_per_band, start_bin)
B, T, F = 16, 1000, 257
N_CORES = 8
TOK = B * T // N_CORES  # tokens per core = 2000
NB = sum(n for n, _, _ in GROUPS)  # 28 bands
OUT_COLS = NB * D  # 3584
P = 128
N_TILES = (TOK + P - 1) // P  # 16 (last tile holds 80 tokens)
N_CHUNK = 512  # matmul free-dim chunk (one PSUM bank)
N_CHUNKS = OUT_COLS // N_CHUNK  # 7
# Per-band layout: (ktile, krow0, c) per band; ktile 0 = bins 0..127,
# ktile 1 = bins 128..255. Output cols for band i are [i*128, (i+1)*128).
_BANDS = []
for _n, _c, _s in GROUPS:
    for _k in range(_n):
        _bin0 = _s + _k * _c
        _BANDS.append((_bin0 // 128, _bin0 % 128, _c))

# x prefetch batches (start tile, ntiles): tiles 0-1 alone so compute
# can start as soon as their columns land.
_XBATCH = [(0, 2), (2, 7), (9, 7)]
# Normalize the first tiles on the (faster, otherwise idle) vector
# engine: the gpsimd queue then opens with tile N's work and the first
# transposes don't sit behind a serial gpsimd chain.
_VEC_NORM_TILES = 3

_STATE = {}


def _build(has_bias):
    """Trace + compile the Bass kernel (cached per process)."""
    from contextlib import ExitStack

    import concourse.bass as bass
    import concourse.tile as tile
    from concourse import bacc, mybir

    f32 = mybir.dt.float32
    f32r = mybir.dt.float32r
    nc = bacc.Bacc(
        "TRN2", target_bir_lowering=False, debug=False, num_devices=N_CORES
    )
    # x arrives already in the SBUF image layout [128, 16*257].
    x_d = nc.dram_tensor("xp", [P, N_TILES * F], f32, kind="ExternalInput").ap()
    # Declared float32r (same 4-byte layout): DMA straight to the fp32r
    # weight tile with no on-chip rounding pass.
    w_d = nc.dram_tensor("wpack", [P, OUT_COLS], f32r, kind="ExternalInput").ap()
    id_d = nc.dram_tensor("ident", [P, P], f32, kind="ExternalInput").ap()
    if has_bias:
        b_d = nc.dram_tensor("bias", [1, OUT_COLS], f32, kind="ExternalInput").ap()
    out_d = nc.dram_tensor("out", [TOK, OUT_COLS], f32, kind="ExternalOutput").ap()

    with tile.TileContext(nc) as tc, ExitStack() as ctx:
        const = ctx.enter_context(tc.tile_pool(name="const", bufs=1))
        xin = ctx.enter_context(tc.tile_pool(name="xin", bufs=1))
        sqp = ctx.enter_context(tc.tile_pool(name="sqp", bufs=3))
        ln = ctx.enter_context(tc.tile_pool(name="ln", bufs=3))
        xnt = ctx.enter_context(tc.tile_pool(name="xnt", bufs=3))
        outp = ctx.enter_context(tc.tile_pool(name="outp", bufs=5))
        ps_tr = ctx.enter_context(tc.tile_pool(name="ps_tr", bufs=2, space="PSUM"))
        ps_mm = ctx.enter_context(tc.tile_pool(name="ps_mm", bufs=3, space="PSUM"))

        # All of x stays resident (16.4 KB/partition): tile t of 128
        # tokens lives at xall[:, t, :].
        xall = xin.tile([P, N_TILES, F], f32)

        # Scalar HWDGE queue, in need order: tiles 0-1, identity, first
        # weight pieces, the rest of x, trailing weight pieces.
        def load_x(t0, nt):
            nc.scalar.dma_start(
                out=xall[:, t0 : t0 + nt, :],
                in_=x_d[:, t0 * F : (t0 + nt) * F].rearrange(
                    "p (a f) -> p a f", a=nt
                ),
            )

        load_x(*_XBATCH[0])
        ident = const.tile([P, P], f32)
        nc.scalar.dma_start(out=ident[:], in_=id_d)
        w_sbr = const.tile([P, OUT_COLS], f32r)
        nc.scalar.dma_start(out=w_sbr[:, 0:1024], in_=w_d[:, 0:1024])
        nc.scalar.dma_start(out=w_sbr[:, 1024:2048], in_=w_d[:, 1024:2048])
        load_x(*_XBATCH[1])
        nc.scalar.dma_start(out=w_sbr[:, 2048:3072], in_=w_d[:, 2048:3072])
        nc.scalar.dma_start(out=w_sbr[:, 3072:3584], in_=w_d[:, 3072:3584])
        load_x(*_XBATCH[2])
        if has_bias:
            bias_sb = const.tile([P, OUT_COLS], f32)
            nc.scalar.dma_start(
                out=bias_sb[:],
                in_=bass.AP(
                    tensor=b_d.tensor, offset=b_d.offset, ap=[[0, P], b_d.ap[1]]
                ),
            )

        eps_t = const.tile([P, 1], f32)
        nc.vector.memset(eps_t[:], EPS)
        # 1/c per band (twice: for sums and sumsq): three constants in a
        # fixed band pattern — built with memsets, no DMA on this path.
        cinv2 = const.tile([P, 2, NB], f32)
        b0 = 0
        for n, c, _s in GROUPS:
            nc.vector.memset(cinv2[:, :, b0 : b0 + n], 1.0 / c)
            b0 += n
        cinv2 = cinv2.rearrange("p a b -> p (a b)")

        for it in range(N_TILES):
            t0 = it * P
            tn = min(P, TOK - t0)

            xt = xall[:tn, it, :]
            norm_eng = nc.vector if it < _VEC_NORM_TILES else nc.gpsimd

            # --- layernorm statistics (per token x band) ---
            sq = sqp.tile([P, 256], f32)
            nc.gpsimd.tensor_mul(sq[:tn, :], xt[:, 0:256], xt[:, 0:256])

            ss = ln.tile([P, 2, NB], f32)
            b0 = 0
            for n, c, s in GROUPS:
                xg = xt[:, s : s + n * c].rearrange("p (g c) -> p g c", g=n)
                sg = sq[:tn, s : s + n * c].rearrange("p (g c) -> p g c", g=n)
                nc.vector.reduce_sum(
                    out=ss[:tn, 0, b0 : b0 + n], in_=xg, axis=mybir.AxisListType.X
                )
                nc.vector.reduce_sum(
                    out=ss[:tn, 1, b0 : b0 + n], in_=sg, axis=mybir.AxisListType.X
                )
                b0 += n

            me = ln.tile([P, 2, NB], f32)  # me[:,0]=mean, me[:,1]=E[x^2]
            nc.vector.tensor_mul(
                me[:tn].rearrange("p a b -> p (a b)"),
                ss[:tn].rearrange("p a b -> p (a b)"),
                cinv2[:tn],
            )
            mean = me[:, 0]
            var = ln.tile([P, NB], f32)
            nc.vector.tensor_mul(var[:tn, :], mean[:tn, :], mean[:tn, :])
            nc.vector.tensor_sub(var[:tn, :], me[:tn, 1, :], var[:tn, :])
            rstd = ln.tile([P, NB], f32)
            nc.scalar.activation(
                out=rstd[:tn, :],
                in_=var[:tn, :],
                func=mybir.ActivationFunctionType.Sqrt,
                bias=eps_t[:tn, :],
                scale=1.0,
            )
            nc.vector.reciprocal(out=rstd[:tn, :], in_=rstd[:tn, :])

            # --- normalize in place: xn = (x - mean) * rstd ---
            b0 = 0
            for n, c, s in GROUPS:
                xg = xt[:, s : s + n * c].rearrange("p (g c) -> p g c", g=n)
                norm_eng.tensor_sub(
                    xg, xg, mean[:tn, b0 : b0 + n].to_broadcast((tn, n, c))
                )
                norm_eng.tensor_mul(
                    xg, xg, rstd[:tn, b0 : b0 + n].to_broadcast((tn, n, c))
                )
                b0 += n

            # --- transpose to [bins, tok] (two 128-col halves) ---
            xnt_h = []
            for h in range(2):
                pt = ps_tr.tile([P, P], f32, tag="pt")
                nc.tensor.transpose(
                    pt[:, :tn], xt[:, h * P : (h + 1) * P], ident[:tn, :tn]
                )
                st = xnt.tile([P, P], f32r, tag=f"xnt{h}")
                nc.scalar.copy(st[:, :tn], pt[:, :tn])
                xnt_h.append(st)

            # --- 7 fp32r matmuls in 2-bank PSUM pairs + drains ---
            # one output store per 2 pairs; sync queue carries only stores.
            ot = outp.tile([P, OUT_COLS], f32)
            for pair in range(4):
                js = [j for j in (2 * pair, 2 * pair + 1) if j < N_CHUNKS]
                pm = ps_mm.tile([P, 2 * N_CHUNK], f32, tag="pm")
                for k, j in enumerate(js):
                    lhsT = xnt_h[0] if j * N_CHUNK < 2560 else xnt_h[1]
                    wcol = j * N_CHUNK
                    nc.tensor.matmul(
                        pm[:tn, k * N_CHUNK : (k + 1) * N_CHUNK],
                        lhsT[:, :tn],
                        w_sbr[:, wcol : wcol + N_CHUNK],
                        start=True,
                        stop=True,
                    )
                c0 = 2 * pair * N_CHUNK
                c1 = c0 + len(js) * N_CHUNK
                osl = ot[:tn, c0:c1]
                pms = pm[:tn, 0 : (c1 - c0)]
                if has_bias:
                    nc.vector.tensor_add(osl, pms, bias_sb[:tn, c0:c1])
                elif pair % 2 == 0:
                    nc.scalar.copy(osl, pms)
                else:
                    nc.vector.tensor_copy(osl, pms)
                if pair % 2 == 1:
                    h0 = (pair - 1) * 2 * N_CHUNK
                    nc.sync.dma_start(
                        out=out_d[t0 : t0 + tn, h0:c1], in_=ot[:tn, h0:c1]
                    )

    nc.compile()
    return nc


def _get_nc(has_bias):
    key = ("nc", has_bias)
    if key not in _STATE:
        _STATE[key] = _build(has_bias)
    return _STATE[key]


def _pack_weights(inputs):
    """Fold gamma into W, beta/b into bias; pack block-diagonal [128, 3584]."""
    wpack = np.zeros((P, OUT_COLS), dtype=np.float32)
    bias = np.zeros((OUT_COLS,), dtype=np.float32)
    bi = 0
    for gi, (n, c, _s) in enumerate(GROUPS, start=1):
        gamma = np.asarray(inputs[f"gamma{gi}"], dtype=np.float32)  # [n, c]
        beta = np.asarray(inputs[f"beta{gi}"], dtype=np.float32)  # [n, c]
        W = np.asarray(inputs[f"W{gi}"], dtype=np.float32)  # [n, c, D]
        b = np.asarray(inputs[f"b{gi}"], dtype=np.float32)  # [n, D]
        for k in range(n):
            _ktile, krow0, cc = _BANDS[bi]
            assert cc == c
            c0, c1 = bi * D, (bi + 1) * D
            wpack[krow0 : krow0 + c, c0:c1] = gamma[k][:, None] * W[k]
            bias[c0:c1] = beta[k] @ W[k] + b[k]
            bi += 1
    return wpack, bias


def _pack_x(xflat):
    """[2000, 257] token-major -> SBUF image [128, 16*257]."""
    xp = np.zeros((P, N_TILES, F), dtype=np.float32)
    full = (TOK // P) * P  # 1920
    xp[:, : TOK // P, :] = xflat[:full].reshape(TOK // P, P, F).transpose(1, 0, 2)
    xp[: TOK - full, TOK // P, :] = xflat[full:]
    return np.ascontiguousarray(xp.reshape(P, N_TILES * F))


def _prepare(inputs):
    """-> (nc, in_maps) for the 8 cores."""
    x = np.asarray(inputs["inputs"], dtype=np.float32)
    assert x.shape == (B, T, F), x.shape
    wpack, bias = _pack_weights(inputs)
    has_bias = bool(np.any(bias != 0.0))

    nc = _get_nc(has_bias)

    xflat = np.ascontiguousarray(x.reshape(B * T, F))
    ident = np.eye(P, dtype=np.float32)
    in_maps = []
    for c in range(N_CORES):
        m = {
            "xp": _pack_x(xflat[c * TOK : (c + 1) * TOK]),
            "wpack": wpack,
            "ident": ident,
        }
        if has_bias:
            m["bias"] = bias.reshape(1, OUT_COLS)
        in_maps.append(m)
    return nc, in_maps


def kernel(**inputs):
    from concourse.bass_utils import run_bass_kernel_spmd

    nc, in_maps = _prepare(inputs)
    res = run_bass_kernel_spmd(nc, in_maps, list(range(N_CORES))).results
    out = np.concatenate([r["out"] for r in res], axis=0)
    return out.reshape(B, T, NB, D)
